# revision 3
# baseline (speedup 1.0000x reference)
"""BiLSTM + pairwise MLP kernel for 8 TRN2 NeuronCores.

Strategy: replicate the (inherently serial) BiLSTM on all 8 cores; shard the
512x512 pair grid row-wise (64 i-rows per core). No collectives needed - each
core writes its own [64*512, 50] output shard, gathered on host.

All weight layout transforms (transposes, gate reorder, bf16 casts) are done
host-side so the device graph is identical across cores (SPMD); the only
per-core input is a one-hot column-selection matrix `sel`.
"""

import sys

sys.path.insert(0, "/opt/trn_rl_repo")

import numpy as np
import ml_dtypes

import concourse.bass as bass
import concourse.bacc as bacc
import concourse.mybir as mybir
import concourse.tile as tile
from concourse.bass_utils import run_bass_kernel_spmd

N = 512
DIN = 300
H = 256
G4 = 4 * H  # 1024
L = 50
NCORES = 8
ISL = N // NCORES  # 64 i-rows per core

BF16 = mybir.dt.bfloat16
F32 = mybir.dt.float32

# test-harness knobs (harness calls kernel() directly; these stay default)
TRACE = False
LAST_EXEC_NS = None
LAST_TRACE_PATH = None
AF = mybir.ActivationFunctionType
ALU = mybir.AluOpType
AX = mybir.AxisListType

# gate order in reference (PyTorch): i, f, g, o  -> reorder to i, f, o, g so
# sigmoid covers [0:768] contiguously and tanh covers [768:1024].
_PERM = np.concatenate(
    [np.arange(0, 256), np.arange(256, 512), np.arange(768, 1024), np.arange(512, 768)]
)
# new layout: i=[0:256], f=[256:512], o=[512:768], g=[768:1024]


def _bf(x):
    return np.ascontiguousarray(x).astype(ml_dtypes.bfloat16)


def _f32(x):
    return np.ascontiguousarray(x).astype(np.float32)


def _chunked(mat, pchunks, free):
    """[pchunks*128, free] -> [128, pchunks*free] with chunk c at cols [c*free:(c+1)*free]."""
    assert mat.shape == (pchunks * 128, free)
    return mat.reshape(pchunks, 128, free).transpose(1, 0, 2).reshape(128, pchunks * free)


def _prep_inputs(x, Wih_f, Whh_f, bih_f, bhh_f, Wih_b, Whh_b, bih_b, bhh_b,
                 W1, b1, W2, b2, W3, b3):
    """Host-side layout prep. Returns dict of device input arrays (shared across cores)."""
    ins = {}
    # recurrent weights: WhhT = Whh[perm].T  [256, 1024] -> [128, 2*1024] bf16
    for nm, Whh in (("whhf", Whh_f), ("whhb", Whh_b)):
        ins[nm] = _bf(_chunked(Whh[_PERM].T, 2, G4))
    # input projection (augmented with bias row, padded K 301->384):
    for nm, Wih, bi, bh in (("wihf", Wih_f, bih_f, bhh_f), ("wihb", Wih_b, bih_b, bhh_b)):
        aug = np.concatenate([Wih[_PERM].T, (bi + bh)[_PERM][None, :]], axis=0)  # [301,1024]
        aug = np.pad(aug, ((0, 384 - 301), (0, 0)))
        ins[nm] = _bf(_chunked(aug, 3, G4))
    # x^T augmented with ones row, padded to 384: [384, 512] -> [128, 3*512]
    xt = np.concatenate([np.asarray(x).T, np.ones((1, N), np.float32)], axis=0)
    xt = np.pad(xt, ((0, 384 - 301), (0, 0)))
    ins["xt"] = _bf(_chunked(xt, 3, N))
    # MLP weights (transposed for lhsT): W1a^T [512,256], W1b^T [512,256]
    ins["w1a"] = _bf(_chunked(W1[:, : 2 * H].T, 4, H))
    ins["w1b"] = _bf(_chunked(W1[:, 2 * H :].T, 4, H))
    ins["w2"] = _bf(_chunked(W2.T, 2, H))
    ins["w3"] = _bf(_chunked(W3.T, 2, L))
    ins["b1"] = _f32(b1.reshape(2, 128).T)  # [128, 2]
    ins["b2"] = _f32(b2.reshape(2, 128).T)
    ins["b3"] = _f32(np.broadcast_to(b3[None, :], (128, L)))  # [128, 50]
    ins["ident"] = _bf(np.eye(128, dtype=np.float32))
    return ins


def _build(tc: tile.TileContext, io: dict):
    nc = tc.nc
    import contextlib

    ctx = contextlib.ExitStack()
    pool = ctx.enter_context(tc.tile_pool(name="persist", bufs=1))

    # ---- load params to SBUF ----
    sb = {}
    for nm in ("whhf", "whhb", "wihf", "wihb", "xt", "w1a", "w1b", "w2", "w3",
               "b1", "b2", "b3", "ident", "sel"):
        ap = io[nm]
        t = pool.tile(list(ap.shape), ap.dtype, tag=nm)
        nc.sync.dma_start(t[:], ap[:])
        sb[nm] = t

    zcol = pool.tile([128, 1], BF16, name="zcol", tag="zcol")
    nc.gpsimd.memset(zcol[:], 0.0)

    # ---- input projections xb_f/xb_b: [512 t, 1024] as 4 chunk tiles each ----
    xb = {"f": [], "b": []}
    xbpool = tc.tile_pool(name="xbpsum", bufs=2, space="PSUM")
    ppool = xbpool.__enter__()
    for d in ("f", "b"):
        wih = sb["wihf" if d == "f" else "wihb"]
        for tch in range(4):
            ps = ppool.tile([128, G4], F32, name="xbps", tag="xbps")
            for nb in range(2):
                for kc in range(3):
                    nc.tensor.matmul(
                        ps[:, nb * 512 : (nb + 1) * 512],
                        sb["xt"][:, kc * N + tch * 128 : kc * N + tch * 128 + 128],
                        wih[:, kc * G4 + nb * 512 : kc * G4 + (nb + 1) * 512],
                        start=(kc == 0), stop=(kc == 2),
                    )
            t = pool.tile([128, G4], BF16, name=f"xb{d}{tch}", tag=f"xb{d}{tch}")
            (nc.scalar.activation(t[:], ps[:], AF.Copy) if tch % 2 == 0
             else nc.vector.tensor_copy(t[:], ps[:]))
            xb[d].append(t)

    xbpool.__exit__(None, None, None)
    tc.strict_bb_all_engine_barrier()

    # ---- outT: LSTM hidden states, hidden-dim on partitions ----
    # outT_f chunks 0..1 (h_f 256), outT_b chunks 0..1 (h_b 256); col t = time t.
    outT = {d: [pool.tile([128, N], BF16, name=f"outT{d}{c}", tag=f"outT{d}{c}") for c in range(2)]
            for d in ("f", "b")}

    cpool = ctx.enter_context(tc.tile_pool(name="cstate", bufs=2))
    gpool = ctx.enter_context(tc.tile_pool(name="gates", bufs=2))
    hpool = ctx.enter_context(tc.tile_pool(name="hstate", bufs=2))
    tpcm = tc.tile_pool(name="tpsum", bufs=2, space="PSUM")
    tppool = tpcm.__enter__()

    c_prev = {}
    for d, row in (("f", 0), ("b", 32)):
        c0 = cpool.tile([33, H], F32, name=f"c{d}", tag=f"c{d}")
        nc.gpsimd.memset(c0[row : row + 1, :], 0.0)
        c_prev[d] = c0

    # ---- LSTM: 512 steps, fwd (t=s) and bwd (t=511-s) interleaved ----
    for s in range(N):
        h_both = hpool.tile([33, H], BF16, name="h", tag="h")
        for d, row in (("f", 0), ("b", 32)):
            t_idx = s if d == "f" else N - 1 - s
            whh = sb["whhf" if d == "f" else "whhb"]
            g = tppool.tile([33, G4], F32, name=f"g{d}", tag=f"g{d}", bufs=1)
            # recurrent matmuls: lhsT = previous hidden column (or zeros at s=0)
            for nb in range(2):
                gr = g[row : row + 1, nb * 512 : (nb + 1) * 512]
                for kc in range(2):
                    if s == 0:
                        lhs = zcol[:, 0:1]
                    else:
                        pc = s - 1 if d == "f" else N - s
                        lhs = outT[d][kc][:, pc : pc + 1]
                    nc.tensor.matmul(gr, lhs,
                                     whh[:, kc * G4 + nb * 512 : kc * G4 + (nb + 1) * 512],
                                     start=(kc == 0), stop=False)
                # + xb row via identity-column one-hot (K=128)
                nc.tensor.matmul(gr,
                                 sb["ident"][:, t_idx % 128 : t_idx % 128 + 1],
                                 xb[d][t_idx // 128][:, nb * 512 : (nb + 1) * 512],
                                 start=False, stop=True)
            # gates: layout [i f o g]; whole chain stays on partition `row`
            sfio = gpool.tile([33, 768], F32, name=f"sfio{d}", tag=f"sfio{d}")[row : row + 1, :]
            nc.scalar.activation(sfio, g[row : row + 1, 0:768], AF.Sigmoid)
            tg = gpool.tile([33, H], F32, name=f"tg{d}", tag=f"tg{d}")[row : row + 1, :]
            nc.scalar.activation(tg, g[row : row + 1, 768:G4], AF.Tanh)
            ig = gpool.tile([33, H], F32, name=f"ig{d}", tag=f"ig{d}")[row : row + 1, :]
            nc.vector.tensor_tensor(ig, sfio[:, 0:H], tg, ALU.mult)
            fc = gpool.tile([33, H], F32, name=f"fc{d}", tag=f"fc{d}")[row : row + 1, :]
            nc.vector.tensor_tensor(fc, sfio[:, H : 2 * H],
                                    c_prev[d][row : row + 1, :], ALU.mult)
            c_new = cpool.tile([33, H], F32, name=f"c{d}", tag=f"c{d}")
            nc.vector.tensor_tensor(c_new[row : row + 1, :], fc, ig, ALU.add)
            tc_t = gpool.tile([33, H], F32, name=f"tc{d}", tag=f"tc{d}")[row : row + 1, :]
            nc.scalar.activation(tc_t, c_new[row : row + 1, :], AF.Tanh)
            nc.vector.tensor_tensor(h_both[row : row + 1, :], sfio[:, 2 * H : 768],
                                    tc_t, ALU.mult)
            c_prev[d] = c_new
        # transpose h -> columns of outT (both dirs at once via rows {0, 32})
        for kc in range(2):
            hT = tppool.tile([128, 33], BF16, name="hT", tag="hT")
            nc.tensor.transpose(hT[:], h_both[0:33, kc * 128 : (kc + 1) * 128],
                                sb["ident"][0:33, 0:33])
            nc.scalar.activation(outT["f"][kc][:, s : s + 1], hT[:, 0:1], AF.Copy)
            nc.scalar.activation(outT["b"][kc][:, N - 1 - s : N - s], hT[:, 32:33],
                                 AF.Copy)

    tc.strict_bb_all_engine_barrier()

    # ==== MLP phase ====
    # out_nat [512 t, 512 h]: 16 PE transposes of outT
    onat = [pool.tile([128, 2 * H], BF16, name=f"onat{tch}", tag=f"onat{tch}") for tch in range(4)]
    allT = outT["f"] + outT["b"]  # h-chunks 0..3
    for tch in range(4):
        for hc in range(4):
            ps = tppool.tile([128, 128], BF16, name="natT", tag="hT")
            nc.tensor.transpose(ps[:], allT[hc][:, tch * 128 : (tch + 1) * 128],
                                sb["ident"][:])
            (nc.scalar.activation(onat[tch][:, hc * 128 : (hc + 1) * 128], ps[:], AF.Copy)
             if hc % 2 == 0 else
             nc.vector.tensor_copy(onat[tch][:, hc * 128 : (hc + 1) * 128], ps[:]))

    # out_sel = sel.T @ out_nat : [64 ii, 512 h]
    osel_ps = tppool.tile([64, 2 * H], F32, name="oselps", tag="mmps")
    for tch in range(4):
        nc.tensor.matmul(osel_ps[:], sb["sel"][:, tch * ISL : (tch + 1) * ISL],
                         onat[tch][:], start=(tch == 0), stop=(tch == 3))
    osel = pool.tile([64, 2 * H], BF16, name="osel", tag="osel")
    nc.scalar.activation(osel[:], osel_ps[:], AF.Copy)

    # outT_my [512 h, 64]: 4 transposes
    otm = [pool.tile([128, ISL], BF16, name=f"otm{hc}", tag=f"otm{hc}") for hc in range(4)]
    for hc in range(4):
        ps = tppool.tile([128, 64], BF16, name="otmT", tag="hT")
        nc.tensor.transpose(ps[:], osel[0:64, hc * 128 : (hc + 1) * 128],
                            sb["ident"][0:64, 0:64])
        nc.vector.tensor_copy(otm[hc][:], ps[:])

    # aT [256, 64] = W1a^T.T @ outT_my  (per m-chunk)
    aT = []
    for mc in range(2):
        ps = tppool.tile([128, ISL], F32, name="aTps", tag="mmps")
        for hc in range(4):
            nc.tensor.matmul(ps[:], sb["w1a"][:, hc * H + mc * 128 : hc * H + mc * 128 + 128],
                             otm[hc][:], start=(hc == 0), stop=(hc == 3))
        t = pool.tile([128, ISL], F32, name=f"aT{mc}", tag=f"aT{mc}")
        nc.scalar.activation(t[:], ps[:], AF.Copy)
        aT.append(t)

    # bT [256, 512] = W1b^T.T @ outT (+ b1), kept bf16
    bT = []
    for mc in range(2):
        ps = tppool.tile([128, N], F32, name="bTps", tag="mmps")
        for hc in range(4):
            nc.tensor.matmul(ps[:], sb["w1b"][:, hc * H + mc * 128 : hc * H + mc * 128 + 128],
                             allT[hc][:], start=(hc == 0), stop=(hc == 3))
        t = pool.tile([128, N], BF16, name=f"bT{mc}", tag=f"bT{mc}")
        nc.scalar.activation(t[:], ps[:], AF.Identity, bias=sb["b1"][:, mc : mc + 1])
        bT.append(t)

    tpcm.__exit__(None, None, None)
    tc.strict_bb_all_engine_barrier()

    # ---- per-i MLP ----
    mpool = ctx.enter_context(tc.tile_pool(name="mlp", bufs=3))
    mps = ctx.enter_context(tc.tile_pool(name="mlpps", bufs=2, space="PSUM"))
    for ii in range(ISL):
        # h1 = relu(bT + aT[:, ii])
        h1 = [mpool.tile([128, N], BF16, name=f"h1{mc}", tag=f"h1{mc}") for mc in range(2)]
        for mc in range(2):
            nc.vector.tensor_scalar(h1[mc][:], bT[mc][:], aT[mc][:, ii : ii + 1], 0.0,
                                    ALU.add, ALU.max)
        # h2 = relu(W2 @ h1 + b2)
        h2ps = [mps.tile([128, N], F32, name=f"h2ps{mc}", tag=f"h2ps{mc}") for mc in range(2)]
        for mc in range(2):
            for kc in range(2):
                nc.tensor.matmul(h2ps[mc][:],
                                 sb["w2"][:, kc * H + mc * 128 : kc * H + mc * 128 + 128],
                                 h1[kc][:], start=(kc == 0), stop=(kc == 1))
        h2s = [mpool.tile([128, N], BF16, name=f"h2s{mc}", tag=f"h2s{mc}") for mc in range(2)]
        for mc in range(2):
            nc.scalar.activation(h2s[mc][:], h2ps[mc][:], AF.Relu,
                                 bias=sb["b2"][:, mc : mc + 1])
        # logits [512 j, 50]: lhsT = h2s chunks, rhs = w3
        lg = mps.tile([128, 4 * L], F32, name="lg", tag="lg")
        for jc in range(4):
            for mc in range(2):
                nc.tensor.matmul(lg[:, jc * L : (jc + 1) * L],
                                 h2s[mc][:, jc * 128 : (jc + 1) * 128],
                                 sb["w3"][:, mc * L : (mc + 1) * L],
                                 start=(mc == 0), stop=(mc == 1))
        # log-softmax over L=50 (free axis); logits bounded -> skip max-subtract
        lb = mpool.tile([128, 4 * L], F32, name="lb", tag="lb")
        for jc in range(4):
            nc.vector.tensor_tensor(lb[:, jc * L : (jc + 1) * L],
                                    lg[:, jc * L : (jc + 1) * L], sb["b3"][:], ALU.add)
        ex = mpool.tile([128, 4 * L], F32, name="ex", tag="ex")
        nc.scalar.activation(ex[:], lb[:], AF.Exp)
        se = mpool.tile([128, 4], F32, name="se", tag="se")
        nc.vector.reduce_sum(se[:], ex[:].rearrange("p (c l) -> p c l", l=L), axis=AX.X)
        ls = mpool.tile([128, 4], F32, name="ls", tag="ls")
        nc.scalar.activation(ls[:], se[:], AF.Ln)
        fin = mpool.tile([128, 4 * L], F32, name="fin", tag="fin")
        for jc in range(4):
            nc.vector.tensor_scalar(fin[:, jc * L : (jc + 1) * L],
                                    lb[:, jc * L : (jc + 1) * L],
                                    ls[:, jc : jc + 1], None, ALU.subtract)
        dst = io["out"][ii * N : (ii + 1) * N, :].rearrange("(c p) l -> p c l", p=128)
        nc.sync.dma_start(dst, fin[:].rearrange("p (c l) -> p c l", l=L))

    ctx.close()


def kernel(**inputs):
    inputs = {k: np.asarray(v) for k, v in inputs.items()}
    ins = _prep_inputs(**inputs)

    nc = bacc.Bacc("TRN2", target_bir_lowering=False, debug=False, num_devices=NCORES)
    io = {}
    for nm, arr in ins.items():
        io[nm] = nc.dram_tensor(nm, list(arr.shape), mybir.dt.from_np(arr.dtype),
                                kind="ExternalInput").ap()
    sel_shape = [128, 4 * ISL]
    io["sel"] = nc.dram_tensor("sel", sel_shape, BF16, kind="ExternalInput").ap()
    io["out"] = nc.dram_tensor("out", [ISL * N, L], F32, kind="ExternalOutput").ap()

    with tile.TileContext(nc) as tcx:
        _build(tcx, io)
    nc.compile()

    in_maps = []
    for cid in range(NCORES):
        m = dict(ins)
        sel = np.zeros((N, ISL), np.float32)
        sel[np.arange(cid * ISL, (cid + 1) * ISL), np.arange(ISL)] = 1.0
        m["sel"] = _bf(_chunked(sel, 4, ISL))
        in_maps.append(m)

    global LAST_EXEC_NS, LAST_TRACE_PATH
    res = run_bass_kernel_spmd(nc, in_maps, core_ids=list(range(NCORES)),
                               trace=TRACE)
    LAST_EXEC_NS = res.exec_time_ns
    if res.instructions_and_trace is not None:
        LAST_TRACE_PATH = res.instructions_and_trace[1]
    out = np.concatenate([res.results[c]["out"] for c in range(NCORES)], axis=0)
    return out


if __name__ == "__main__":
    # smoke test with random inputs
    rng = np.random.default_rng(0)
    s = 1.0 / np.sqrt(H)
    ins = {
        "x": rng.standard_normal((N, DIN)).astype(np.float32),
    }
    for nm, shape in [("Wih_f", (G4, DIN)), ("Whh_f", (G4, H)), ("bih_f", (G4,)),
                      ("bhh_f", (G4,)), ("Wih_b", (G4, DIN)), ("Whh_b", (G4, H)),
                      ("bih_b", (G4,)), ("bhh_b", (G4,)), ("W1", (H, G4)),
                      ("b1", (H,)), ("W2", (H, H)), ("b2", (H,)), ("W3", (L, H)),
                      ("b3", (L,))]:
        ins[nm] = (rng.uniform(-s, s, shape)).astype(np.float32)
    out = kernel(**ins)
    print(out.shape, out.dtype, np.isfinite(out).all())

